# revision 6
# baseline (speedup 1.0000x reference)
"""Multi-head self-attention Trainium2 kernel (8 NeuronCores, SPMD).

Problem: x[2, 4096, 512], 8 heads, dq=64.
  q = x@Wq.T+bq ; k = x@Wk.T+bk ; v = x@Wv.T+bv
  att = softmax(q k^T / sqrt(64)) ; out = (att v) @ Wo.T + bo

Sharding: batch (2) x query-quarter (4) -> 8 cores. Core c handles batch
c//4, query rows [(c%4)*1024, (c%4+1)*1024). Every core computes full K/V
for its batch (duplicated 4x, cheap), attention for all 8 heads over its
1024 query rows, and the output projection for its rows. The host only
concatenates per-core outputs -- no cross-core reduction needed.

On-device layout notes (per core):
  - All matmuls contract over the partition dim, so x and the weights are
    PE-transposed once into SBUF at kernel start (bf16).
  - Scores are computed transposed, S.T[tk, tq], per head: lhsT = K_h.T
    (dq=64 contraction; even/odd heads live on partitions 0-63/64-127 so
    consecutive matmuls occupy disjoint PE row-groups and overlap).
  - exp runs on ScalarE straight out of multi-bank PSUM into SBUF (bf16),
    with the 1/sqrt(dq) scale folded into the activation's scale. This is
    the kernel's bottleneck engine (~33.5M exps per core).
  - att@V uses V with an appended ones-column: the extra output row is the
    softmax denominator, so no separate reduction pass is needed.
  - Normalization: VectorE reciprocal of the denominator row + GpSimd
    partition-broadcast + VectorE multiply.
"""

import numpy as np

import concourse.bass as bass
import concourse.bacc as bacc
import concourse.mybir as mybir
import concourse.tile as tile
from concourse.bass_utils import run_bass_kernel_spmd
from concourse.masks import make_identity

F32 = mybir.dt.float32
BF16 = mybir.dt.bfloat16

B = 2
T = 4096
D = 512
H = 8
DQ = 64
TQ = 1024  # query rows per core
NCORES = 8
NHP = 4  # head pairs
G = 3  # tk tiles (of 128) per exp group: 3 PSUM banks per score tile


def _build_program():
    nc = bacc.Bacc(None)

    xkv = nc.declare_dram_parameter("xkv", [T, D], F32, isOutput=False)
    xq = nc.declare_dram_parameter("xq", [TQ, D], F32, isOutput=False)
    ws = {
        name: nc.declare_dram_parameter(name, [D, D], F32, isOutput=False)
        for name in ("Wq", "Wk", "Wv", "Wo")
    }
    bs = {
        name: nc.declare_dram_parameter(name, [D], F32, isOutput=False)
        for name in ("bq", "bk", "bv", "bo")
    }
    y = nc.declare_dram_parameter("y", [TQ, D], F32, isOutput=True)

    with tile.TileContext(nc) as tc:
        _emit(nc, tc, xkv, xq, ws, bs, y)
    if not nc.is_finalized():
        nc.finalize()
    return nc


def _emit(nc, tc, xkv, xq, ws, bs, y):
    from contextlib import ExitStack

    ctx = ExitStack()
    with ctx:
        persist = ctx.enter_context(tc.tile_pool(name="persist", bufs=1))

        # persistent SBUF tensors
        XT = persist.tile([128, 4, T], BF16)  # x[batch].T  (d on partitions)
        XQT = persist.tile([128, 4, TQ], BF16)  # xq.T
        WqT = persist.tile([128, 4, D], BF16)
        WkT = persist.tile([128, 4, D], BF16)
        WvT = persist.tile([128, 4, D], BF16)
        WoT = persist.tile([128, 4, D], BF16)
        KT = persist.tile([128, NHP, T], BF16)  # K.T per head-pair
        QT = persist.tile([128, NHP, TQ], BF16)
        VH = persist.tile([128, T // 128, H, DQ + 1], BF16)  # [V | ones]
        AOT = persist.tile([128, NHP, TQ], BF16)  # normalized (att@V).T
        bq_s = persist.tile([128, 4], F32)
        bk_s = persist.tile([128, 4], F32)
        bv_b = persist.tile([128, D], F32)
        bo_b = persist.tile([128, D], F32)
        identity = persist.tile([128, 128], F32)

        make_identity(nc, identity)
        # per-partition bias layouts for Q.T/K.T (bias indexed by j = partition)
        nc.sync.dma_start(out=bq_s, in_=bs["bq"].rearrange("(a p) -> p a", p=128))
        nc.sync.dma_start(out=bk_s, in_=bs["bk"].rearrange("(a p) -> p a", p=128))
        # broadcast bias layouts for V / y (bias indexed by j = free dim)
        nc.sync.dma_start(
            out=bv_b,
            in_=bs["bv"].rearrange("(a d) -> a d", a=1).to_broadcast((128, D)),
        )
        nc.sync.dma_start(
            out=bo_b,
            in_=bs["bo"].rearrange("(a d) -> a d", a=1).to_broadcast((128, D)),
        )
        nc.vector.memset(VH[:, :, :, DQ : DQ + 1], 1.0)

        wts = {"Wq": WqT, "Wk": WkT, "Wv": WvT, "Wo": WoT}

        # ---- Stage A/B: transpose weights and x into SBUF (PE transpose) ----
        with (
            tc.tile_pool(name="ld", bufs=3) as ld,
            tc.tile_pool(name="tp", bufs=8, space="PSUM") as tp,
        ):
            for name, wt in wts.items():
                for jt in range(4):
                    wblk = ld.tile([128, D], F32, tag="blk")
                    nc.sync.dma_start(out=wblk, in_=ws[name][jt * 128 : (jt + 1) * 128, :])
                    for dc in range(4):
                        pt = tp.tile([128, 128], F32, tag="tr")
                        nc.tensor.transpose(pt, wblk[:, dc * 128 : (dc + 1) * 128], identity)
                        nc.vector.tensor_copy(out=wt[:, dc, jt * 128 : (jt + 1) * 128], in_=pt)
            for tt in range(T // 128):
                xblk = ld.tile([128, D], F32, tag="blk")
                nc.sync.dma_start(out=xblk, in_=xkv[tt * 128 : (tt + 1) * 128, :])
                for dc in range(4):
                    pt = tp.tile([128, 128], F32, tag="tr")
                    nc.tensor.transpose(pt, xblk[:, dc * 128 : (dc + 1) * 128], identity)
                    nc.vector.tensor_copy(out=XT[:, dc, tt * 128 : (tt + 1) * 128], in_=pt)
            for tt in range(TQ // 128):
                xblk = ld.tile([128, D], F32, tag="blk")
                nc.sync.dma_start(out=xblk, in_=xq[tt * 128 : (tt + 1) * 128, :])
                for dc in range(4):
                    pt = tp.tile([128, 128], F32, tag="tr")
                    nc.tensor.transpose(pt, xblk[:, dc * 128 : (dc + 1) * 128], identity)
                    nc.vector.tensor_copy(out=XQT[:, dc, tt * 128 : (tt + 1) * 128], in_=pt)

        # ---- Stage C: projections ----
        with tc.tile_pool(name="pp", bufs=6, space="PSUM") as pp:
            # K.T[j, t] and Q.T[j, t]: lhsT = W.T[d, j-tile], rhs = X.T[d, t-chunk]
            for jt in range(4):
                for ch in range(T // 512):
                    kp = pp.tile([128, 512], F32, tag="proj")
                    for dc in range(4):
                        nc.tensor.matmul(
                            kp,
                            lhsT=WkT[:, dc, jt * 128 : (jt + 1) * 128],
                            rhs=XT[:, dc, ch * 512 : (ch + 1) * 512],
                            start=(dc == 0),
                            stop=(dc == 3),
                        )
                    nc.vector.tensor_scalar_add(
                        out=KT[:, jt, ch * 512 : (ch + 1) * 512],
                        in0=kp,
                        scalar1=bk_s[:, jt : jt + 1],
                    )
                for ch in range(TQ // 512):
                    qp = pp.tile([128, 512], F32, tag="proj")
                    for dc in range(4):
                        nc.tensor.matmul(
                            qp,
                            lhsT=WqT[:, dc, jt * 128 : (jt + 1) * 128],
                            rhs=XQT[:, dc, ch * 512 : (ch + 1) * 512],
                            start=(dc == 0),
                            stop=(dc == 3),
                        )
                    nc.vector.tensor_scalar_add(
                        out=QT[:, jt, ch * 512 : (ch + 1) * 512],
                        in0=qp,
                        scalar1=bq_s[:, jt : jt + 1],
                    )
            # V[t, j]: lhsT = X.T[d, t-tile], rhs = Wv.T[d, :]
            for tt in range(T // 128):
                vp = pp.tile([128, 512], F32, tag="proj")
                for dc in range(4):
                    nc.tensor.matmul(
                        vp,
                        lhsT=XT[:, dc, tt * 128 : (tt + 1) * 128],
                        rhs=WvT[:, dc, :],
                        start=(dc == 0),
                        stop=(dc == 3),
                    )
                nc.vector.tensor_add(
                    out=VH[:, tt, :, 0:DQ],
                    in0=vp.rearrange("p (h v) -> p h v", h=H),
                    in1=bv_b.rearrange("p (h v) -> p h v", h=H),
                )

        # ---- Stage D: attention ----
        ngroups = (T // 128 + G - 1) // G
        with (
            tc.tile_pool(name="sp", bufs=2, space="PSUM") as sp,
            tc.tile_pool(name="op", bufs=2, space="PSUM") as op,
            tc.tile_pool(name="se", bufs=3) as se,
            tc.tile_pool(name="epi", bufs=4) as epi,
        ):
            for qc in range(TQ // 512):
                qsl = slice(qc * 512, (qc + 1) * 512)
                for hp in range(NHP):
                    attO = [
                        op.tile([DQ + 1, 512], F32, tag="attO", name=f"attO{p}")
                        for p in range(2)
                    ]
                    tkt = 0
                    for g in range(ngroups):
                        gsz = min(G, T // 128 - tkt)
                        for par in range(2):
                            h = hp * 2 + par
                            psl = slice(par * 64, (par + 1) * 64)
                            sc = sp.tile([128, G, 512], F32, tag="sc")
                            for i in range(gsz):
                                tksl = slice((tkt + i) * 128, (tkt + i + 1) * 128)
                                nc.tensor.matmul(
                                    sc[:, i, :],
                                    lhsT=KT[psl, hp, tksl],
                                    rhs=QT[psl, hp, qsl],
                                    start=True,
                                    stop=True,
                                )
                            ex = se.tile([128, G, 512], BF16, tag="ex")
                            nc.scalar.activation(
                                out=ex[:, :gsz, :],
                                in_=sc[:, :gsz, :],
                                func=mybir.ActivationFunctionType.Exp,
                                scale=float(DQ) ** -0.5,
                            )
                            for i in range(gsz):
                                nc.tensor.matmul(
                                    attO[par],
                                    lhsT=VH[:, tkt + i, h, :],
                                    rhs=ex[:, i, :],
                                    start=(tkt + i == 0),
                                    stop=(tkt + i == T // 128 - 1),
                                    skip_group_check=True,
                                )
                        tkt += gsz
                    for par in range(2):
                        rden = epi.tile([1, 512], F32, tag="rden")
                        nc.vector.reciprocal(out=rden, in_=attO[par][DQ : DQ + 1, :])
                        bc = epi.tile([64, 512], F32, tag="bc")
                        nc.gpsimd.partition_broadcast(out_ap=bc, in_ap=rden)
                        nc.vector.tensor_mul(
                            out=AOT[par * 64 : (par + 1) * 64, hp, qsl],
                            in0=attO[par][0:DQ, :],
                            in1=bc,
                        )

        # ---- Stage E: output projection ----
        with (
            tc.tile_pool(name="yp", bufs=4, space="PSUM") as yp,
            tc.tile_pool(name="yo", bufs=4) as yo,
        ):
            for tt in range(TQ // 128):
                ypt = yp.tile([128, 512], F32, tag="y")
                for hp in range(NHP):
                    nc.tensor.matmul(
                        ypt,
                        lhsT=AOT[:, hp, tt * 128 : (tt + 1) * 128],
                        rhs=WoT[:, hp, :],
                        start=(hp == 0),
                        stop=(hp == NHP - 1),
                    )
                ys = yo.tile([128, 512], F32, tag="ys")
                nc.vector.tensor_add(out=ys, in0=ypt, in1=bo_b)
                nc.sync.dma_start(out=y[tt * 128 : (tt + 1) * 128, :], in_=ys)


_CACHED = {}


def _get_program():
    if "nc" not in _CACHED:
        _CACHED["nc"] = _build_program()
    return _CACHED["nc"]


def kernel(x, Wq, bq, Wk, bk, Wv, bv, Wo, bo, _trace=False):
    x = np.ascontiguousarray(np.asarray(x, dtype=np.float32))
    weights = {
        "Wq": np.ascontiguousarray(np.asarray(Wq, dtype=np.float32)),
        "Wk": np.ascontiguousarray(np.asarray(Wk, dtype=np.float32)),
        "Wv": np.ascontiguousarray(np.asarray(Wv, dtype=np.float32)),
        "Wo": np.ascontiguousarray(np.asarray(Wo, dtype=np.float32)),
        "bq": np.ascontiguousarray(np.asarray(bq, dtype=np.float32)),
        "bk": np.ascontiguousarray(np.asarray(bk, dtype=np.float32)),
        "bv": np.ascontiguousarray(np.asarray(bv, dtype=np.float32)),
        "bo": np.ascontiguousarray(np.asarray(bo, dtype=np.float32)),
    }

    nc = _get_program()
    in_maps = []
    for c in range(NCORES):
        b = c // 4
        q0 = (c % 4) * TQ
        in_maps.append(
            {"xkv": x[b], "xq": x[b, q0 : q0 + TQ], **weights}
        )

    res = run_bass_kernel_spmd(
        nc, in_maps, core_ids=list(range(NCORES)), trace=_trace
    )

    out = np.empty((B, T, D), dtype=np.float32)
    for c in range(NCORES):
        b = c // 4
        q0 = (c % 4) * TQ
        out[b, q0 : q0 + TQ] = res.results[c]["y"]
    if _trace:
        return out, res
    return out


# revision 8
# speedup vs baseline: 1.3420x; 1.3420x over previous
"""Multi-head self-attention Trainium2 kernel (8 NeuronCores, SPMD).

Problem: x[2, 4096, 512], 8 heads, dq=64.
  q = x@Wq.T+bq ; k = x@Wk.T+bk ; v = x@Wv.T+bv
  att = softmax(q k^T / sqrt(64)) ; out = (att v) @ Wo.T + bo

Sharding: batch (2) x query-quarter (4) -> 8 cores. Core c handles batch
c//4, query rows [(c%4)*1024, (c%4+1)*1024). Every core computes full K/V
for its batch (duplicated 4x, cheap), attention for all 8 heads over its
1024 query rows, and the output projection for its rows. The host only
concatenates per-core outputs -- no cross-core reduction needed.

On-device layout notes (per core):
  - All matmuls contract over the partition dim, so x and the weights are
    PE-transposed once into SBUF at kernel start (bf16).
  - Scores are computed transposed, S.T[tk, tq], per head: lhsT = K_h.T
    (dq=64 contraction; even/odd heads live on partitions 0-63/64-127 so
    consecutive matmuls occupy disjoint PE row-groups and overlap).
  - exp runs on ScalarE straight out of multi-bank PSUM into SBUF (bf16),
    with the 1/sqrt(dq) scale folded into the activation's scale. This is
    the kernel's bottleneck engine (~33.5M exps per core).
  - att@V uses V with an appended ones-column: the extra output row is the
    softmax denominator, so no separate reduction pass is needed.
  - Normalization: VectorE reciprocal of the denominator row + GpSimd
    partition-broadcast + VectorE multiply.
"""

import numpy as np

import concourse.bass as bass
import concourse.bacc as bacc
import concourse.mybir as mybir
import concourse.tile as tile
from concourse.bass_utils import run_bass_kernel_spmd
from concourse.masks import make_identity

F32 = mybir.dt.float32
BF16 = mybir.dt.bfloat16

B = 2
T = 4096
D = 512
H = 8
DQ = 64
TQ = 1024  # query rows per core
NCORES = 8
NHP = 4  # head pairs
G = 2  # tk tiles (of 128) per exp group: 2 PSUM banks per score tile


def _build_program():
    nc = bacc.Bacc(None)

    xkv = nc.declare_dram_parameter("xkv", [T, D], F32, isOutput=False)
    xq = nc.declare_dram_parameter("xq", [TQ, D], F32, isOutput=False)
    ws = {
        name: nc.declare_dram_parameter(name, [D, D], F32, isOutput=False)
        for name in ("Wq", "Wk", "Wv", "Wo")
    }
    bs = {
        name: nc.declare_dram_parameter(name, [D], F32, isOutput=False)
        for name in ("bq", "bk", "bv", "bo")
    }
    y = nc.declare_dram_parameter("y", [TQ, D], F32, isOutput=True)

    with tile.TileContext(nc) as tc:
        _emit(nc, tc, xkv, xq, ws, bs, y)
    if not nc.is_finalized():
        nc.finalize()
    return nc


def _emit(nc, tc, xkv, xq, ws, bs, y):
    from contextlib import ExitStack

    ctx = ExitStack()
    with ctx:
        persist = ctx.enter_context(tc.tile_pool(name="persist", bufs=1))

        # persistent SBUF tensors
        XT = persist.tile([128, 4, T], BF16)  # x[batch].T  (d on partitions)
        XQT = persist.tile([128, 4, TQ], BF16)  # xq.T
        WqT = persist.tile([128, 4, D], BF16)
        WkT = persist.tile([128, 4, D], BF16)
        WvT = persist.tile([128, 4, D], BF16)
        WoT = persist.tile([128, 4, D], BF16)
        KT = persist.tile([128, NHP, T], BF16)  # K.T per head-pair
        QT = persist.tile([128, NHP, TQ], BF16)
        VH = persist.tile([128, T // 128, H, DQ + 1], BF16)  # [V | ones]
        AOT = persist.tile([128, NHP, TQ], BF16)  # normalized (att@V).T
        bq_s = persist.tile([128, 4], F32)
        bk_s = persist.tile([128, 4], F32)
        bv_b = persist.tile([128, D], F32)
        bo_b = persist.tile([128, D], F32)
        identity = persist.tile([128, 128], F32)

        make_identity(nc, identity)
        # per-partition bias layouts for Q.T/K.T (bias indexed by j = partition)
        nc.sync.dma_start(out=bq_s, in_=bs["bq"].rearrange("(a p) -> p a", p=128))
        nc.sync.dma_start(out=bk_s, in_=bs["bk"].rearrange("(a p) -> p a", p=128))
        # broadcast bias layouts for V / y (bias indexed by j = free dim)
        nc.sync.dma_start(
            out=bv_b,
            in_=bs["bv"].rearrange("(a d) -> a d", a=1).to_broadcast((128, D)),
        )
        nc.sync.dma_start(
            out=bo_b,
            in_=bs["bo"].rearrange("(a d) -> a d", a=1).to_broadcast((128, D)),
        )
        nc.vector.memset(VH[:, :, :, DQ : DQ + 1], 1.0)

        wts = {"Wq": WqT, "Wk": WkT, "Wv": WvT, "Wo": WoT}

        # ---- Stage A/B: transpose weights and x into SBUF (PE transpose) ----
        with (
            tc.tile_pool(name="ld", bufs=3) as ld,
            tc.tile_pool(name="tp", bufs=8, space="PSUM") as tp,
        ):
            for name, wt in wts.items():
                for jt in range(4):
                    wblk = ld.tile([128, D], F32, tag="blk")
                    nc.sync.dma_start(out=wblk, in_=ws[name][jt * 128 : (jt + 1) * 128, :])
                    for dc in range(4):
                        pt = tp.tile([128, 128], F32, tag="tr")
                        nc.tensor.transpose(pt, wblk[:, dc * 128 : (dc + 1) * 128], identity)
                        nc.vector.tensor_copy(out=wt[:, dc, jt * 128 : (jt + 1) * 128], in_=pt)
            for tt in range(T // 128):
                xblk = ld.tile([128, D], F32, tag="blk")
                nc.sync.dma_start(out=xblk, in_=xkv[tt * 128 : (tt + 1) * 128, :])
                for dc in range(4):
                    pt = tp.tile([128, 128], F32, tag="tr")
                    nc.tensor.transpose(pt, xblk[:, dc * 128 : (dc + 1) * 128], identity)
                    nc.vector.tensor_copy(out=XT[:, dc, tt * 128 : (tt + 1) * 128], in_=pt)
            for tt in range(TQ // 128):
                xblk = ld.tile([128, D], F32, tag="blk")
                nc.sync.dma_start(out=xblk, in_=xq[tt * 128 : (tt + 1) * 128, :])
                for dc in range(4):
                    pt = tp.tile([128, 128], F32, tag="tr")
                    nc.tensor.transpose(pt, xblk[:, dc * 128 : (dc + 1) * 128], identity)
                    nc.vector.tensor_copy(out=XQT[:, dc, tt * 128 : (tt + 1) * 128], in_=pt)

        # ---- Stage C: projections ----
        with tc.tile_pool(name="pp", bufs=6, space="PSUM") as pp:
            # K.T[j, t] and Q.T[j, t]: lhsT = W.T[d, j-tile], rhs = X.T[d, t-chunk]
            for jt in range(4):
                for ch in range(T // 512):
                    kp = pp.tile([128, 512], F32, tag="proj")
                    for dc in range(4):
                        nc.tensor.matmul(
                            kp,
                            lhsT=WkT[:, dc, jt * 128 : (jt + 1) * 128],
                            rhs=XT[:, dc, ch * 512 : (ch + 1) * 512],
                            start=(dc == 0),
                            stop=(dc == 3),
                        )
                    nc.vector.tensor_scalar_add(
                        out=KT[:, jt, ch * 512 : (ch + 1) * 512],
                        in0=kp,
                        scalar1=bk_s[:, jt : jt + 1],
                    )
                for ch in range(TQ // 512):
                    qp = pp.tile([128, 512], F32, tag="proj")
                    for dc in range(4):
                        nc.tensor.matmul(
                            qp,
                            lhsT=WqT[:, dc, jt * 128 : (jt + 1) * 128],
                            rhs=XQT[:, dc, ch * 512 : (ch + 1) * 512],
                            start=(dc == 0),
                            stop=(dc == 3),
                        )
                    nc.vector.tensor_scalar_add(
                        out=QT[:, jt, ch * 512 : (ch + 1) * 512],
                        in0=qp,
                        scalar1=bq_s[:, jt : jt + 1],
                    )
            # V[t, j]: lhsT = X.T[d, t-tile], rhs = Wv.T[d, :]
            for tt in range(T // 128):
                vp = pp.tile([128, 512], F32, tag="proj")
                for dc in range(4):
                    nc.tensor.matmul(
                        vp,
                        lhsT=XT[:, dc, tt * 128 : (tt + 1) * 128],
                        rhs=WvT[:, dc, :],
                        start=(dc == 0),
                        stop=(dc == 3),
                    )
                nc.vector.tensor_add(
                    out=VH[:, tt, :, 0:DQ],
                    in0=vp.rearrange("p (h v) -> p h v", h=H),
                    in1=bv_b.rearrange("p (h v) -> p h v", h=H),
                )

        # ---- Stage D: attention ----
        ngroups = (T // 128 + G - 1) // G
        with (
            tc.tile_pool(name="sp", bufs=3, space="PSUM") as sp,
            tc.tile_pool(name="op", bufs=2, space="PSUM") as op,
            tc.tile_pool(name="se", bufs=4) as se,
            tc.tile_pool(name="epi", bufs=4) as epi,
        ):
            for qc in range(TQ // 512):
                qsl = slice(qc * 512, (qc + 1) * 512)
                for hp in range(NHP):
                    attO = [
                        op.tile([DQ + 1, 512], F32, tag="attO", name=f"attO{p}")
                        for p in range(2)
                    ]
                    tkt = 0
                    for g in range(ngroups):
                        gsz = min(G, T // 128 - tkt)
                        sc = [
                            sp.tile([128, G, 512], F32, tag="sc", name=f"sc{p}")
                            for p in range(2)
                        ]
                        # interleave even/odd head score matmuls: adjacent
                        # K=64 matmuls on disjoint PE row-groups overlap
                        for i in range(gsz):
                            tksl = slice((tkt + i) * 128, (tkt + i + 1) * 128)
                            for par in range(2):
                                psl = slice(par * 64, (par + 1) * 64)
                                nc.tensor.matmul(
                                    sc[par][:, i, :],
                                    lhsT=KT[psl, hp, tksl],
                                    rhs=QT[psl, hp, qsl],
                                    start=True,
                                    stop=True,
                                )
                        for par in range(2):
                            h = hp * 2 + par
                            ex = se.tile([128, G, 512], BF16, tag="ex")
                            nc.scalar.activation(
                                out=ex[:, :gsz, :],
                                in_=sc[par][:, :gsz, :],
                                func=mybir.ActivationFunctionType.Exp,
                                scale=float(DQ) ** -0.5,
                            )
                            for i in range(gsz):
                                nc.tensor.matmul(
                                    attO[par],
                                    lhsT=VH[:, tkt + i, h, :],
                                    rhs=ex[:, i, :],
                                    start=(tkt + i == 0),
                                    stop=(tkt + i == T // 128 - 1),
                                    skip_group_check=True,
                                )
                        tkt += gsz
                    for par in range(2):
                        rden = epi.tile([1, 512], F32, tag="rden")
                        nc.vector.reciprocal_approx_fast(
                            out=rden, in_=attO[par][DQ : DQ + 1, :]
                        )
                        bc = epi.tile([64, 512], F32, tag="bc")
                        nc.gpsimd.partition_broadcast(out_ap=bc, in_ap=rden)
                        nc.vector.tensor_mul(
                            out=AOT[par * 64 : (par + 1) * 64, hp, qsl],
                            in0=attO[par][0:DQ, :],
                            in1=bc,
                        )

        # ---- Stage E: output projection ----
        with (
            tc.tile_pool(name="yp", bufs=4, space="PSUM") as yp,
            tc.tile_pool(name="yo", bufs=4) as yo,
        ):
            for tt in range(TQ // 128):
                ypt = yp.tile([128, 512], F32, tag="y")
                for hp in range(NHP):
                    nc.tensor.matmul(
                        ypt,
                        lhsT=AOT[:, hp, tt * 128 : (tt + 1) * 128],
                        rhs=WoT[:, hp, :],
                        start=(hp == 0),
                        stop=(hp == NHP - 1),
                    )
                ys = yo.tile([128, 512], F32, tag="ys")
                nc.vector.tensor_add(out=ys, in0=ypt, in1=bo_b)
                nc.sync.dma_start(out=y[tt * 128 : (tt + 1) * 128, :], in_=ys)


_CACHED = {}


def _get_program():
    if "nc" not in _CACHED:
        _CACHED["nc"] = _build_program()
    return _CACHED["nc"]


def kernel(x, Wq, bq, Wk, bk, Wv, bv, Wo, bo, _trace=False):
    x = np.ascontiguousarray(np.asarray(x, dtype=np.float32))
    weights = {
        "Wq": np.ascontiguousarray(np.asarray(Wq, dtype=np.float32)),
        "Wk": np.ascontiguousarray(np.asarray(Wk, dtype=np.float32)),
        "Wv": np.ascontiguousarray(np.asarray(Wv, dtype=np.float32)),
        "Wo": np.ascontiguousarray(np.asarray(Wo, dtype=np.float32)),
        "bq": np.ascontiguousarray(np.asarray(bq, dtype=np.float32)),
        "bk": np.ascontiguousarray(np.asarray(bk, dtype=np.float32)),
        "bv": np.ascontiguousarray(np.asarray(bv, dtype=np.float32)),
        "bo": np.ascontiguousarray(np.asarray(bo, dtype=np.float32)),
    }

    nc = _get_program()
    in_maps = []
    for c in range(NCORES):
        b = c // 4
        q0 = (c % 4) * TQ
        in_maps.append(
            {"xkv": x[b], "xq": x[b, q0 : q0 + TQ], **weights}
        )

    res = run_bass_kernel_spmd(
        nc, in_maps, core_ids=list(range(NCORES)), trace=_trace
    )

    out = np.empty((B, T, D), dtype=np.float32)
    for c in range(NCORES):
        b = c // 4
        q0 = (c % 4) * TQ
        out[b, q0 : q0 + TQ] = res.results[c]["y"]
    if _trace:
        return out, res
    return out


# revision 11
# speedup vs baseline: 1.4196x; 1.0578x over previous
"""Multi-head self-attention Trainium2 kernel (8 NeuronCores, SPMD).

Problem: x[2, 4096, 512], 8 heads, dq=64.
  q = x@Wq.T+bq ; k = x@Wk.T+bk ; v = x@Wv.T+bv
  att = softmax(q k^T / sqrt(64)) ; out = (att v) @ Wo.T + bo

Sharding: batch (2) x query-quarter (4) -> 8 cores. Core c handles batch
c//4, query rows [(c%4)*1024, (c%4+1)*1024). Every core computes full K/V
for its batch (duplicated 4x, cheap), attention for all 8 heads over its
1024 query rows, and the output projection for its rows. The host only
concatenates per-core outputs -- no cross-core reduction needed.

On-device layout notes (per core):
  - All matmuls contract over the partition dim, so x and the weights are
    PE-transposed once into SBUF at kernel start (bf16).
  - Scores are computed transposed, S.T[tk, tq], per head: lhsT = K_h.T
    (dq=64 contraction; even/odd heads live on partitions 0-63/64-127 so
    consecutive matmuls occupy disjoint PE row-groups and overlap).
  - exp runs on ScalarE straight out of multi-bank PSUM into SBUF (bf16),
    with the 1/sqrt(dq) scale folded into the activation's scale. This is
    the kernel's bottleneck engine (~33.5M exps per core).
  - att@V uses V with an appended ones-column: the extra output row is the
    softmax denominator, so no separate reduction pass is needed.
  - Normalization: VectorE reciprocal of the denominator row + GpSimd
    partition-broadcast + VectorE multiply.
"""

import numpy as np

import concourse.bass as bass
import concourse.bacc as bacc
import concourse.mybir as mybir
import concourse.tile as tile
from concourse.bass_utils import run_bass_kernel_spmd
from concourse.masks import make_identity

F32 = mybir.dt.float32
BF16 = mybir.dt.bfloat16

B = 2
T = 4096
D = 512
H = 8
DQ = 64
TQ = 1024  # query rows per core
NCORES = 8
NHP = 4  # head pairs
G = 2  # tk tiles (of 128) per exp group: 2 PSUM banks per score tile


def _build_program():
    nc = bacc.Bacc(None)

    xkv = nc.declare_dram_parameter("xkv", [T, D], F32, isOutput=False)
    xq = nc.declare_dram_parameter("xq", [TQ, D], F32, isOutput=False)
    ws = {
        name: nc.declare_dram_parameter(name, [D, D], F32, isOutput=False)
        for name in ("Wq", "Wk", "Wv", "Wo")
    }
    bs = {
        name: nc.declare_dram_parameter(name, [D], F32, isOutput=False)
        for name in ("bq", "bk", "bv", "bo")
    }
    y = nc.declare_dram_parameter("y", [TQ, D], F32, isOutput=True)

    with tile.TileContext(nc) as tc:
        _emit(nc, tc, xkv, xq, ws, bs, y)
    if not nc.is_finalized():
        nc.finalize()
    return nc


def _emit(nc, tc, xkv, xq, ws, bs, y):
    from contextlib import ExitStack

    ctx = ExitStack()
    with ctx:
        persist = ctx.enter_context(tc.tile_pool(name="persist", bufs=1))

        # persistent SBUF tensors
        XT = persist.tile([128, 4, T], BF16)  # x[batch].T  (d on partitions)
        XQT = persist.tile([128, 4, TQ], BF16)  # xq.T
        WqT = persist.tile([128, 4, D], BF16)
        WkT = persist.tile([128, 4, D], BF16)
        WvT = persist.tile([128, 4, D], BF16)
        WoT = persist.tile([128, 4, D], BF16)
        KT = persist.tile([128, NHP, T], BF16)  # K.T per head-pair
        QT = persist.tile([128, NHP, TQ], BF16)
        VH = persist.tile([128, T // 128, H, DQ + 1], BF16)  # [V | ones]
        AOT = persist.tile([128, NHP, TQ], BF16)  # normalized (att@V).T
        bq_s = persist.tile([128, 4], F32)
        bk_s = persist.tile([128, 4], F32)
        bv_b = persist.tile([128, D], F32)
        bo_b = persist.tile([128, D], F32)
        identity = persist.tile([128, 128], F32)

        make_identity(nc, identity)
        # per-partition bias layouts for Q.T/K.T (bias indexed by j = partition)
        nc.sync.dma_start(out=bq_s, in_=bs["bq"].rearrange("(a p) -> p a", p=128))
        nc.sync.dma_start(out=bk_s, in_=bs["bk"].rearrange("(a p) -> p a", p=128))
        # broadcast bias layouts for V / y (bias indexed by j = free dim)
        nc.sync.dma_start(
            out=bv_b,
            in_=bs["bv"].rearrange("(a d) -> a d", a=1).to_broadcast((128, D)),
        )
        nc.sync.dma_start(
            out=bo_b,
            in_=bs["bo"].rearrange("(a d) -> a d", a=1).to_broadcast((128, D)),
        )
        nc.vector.memset(VH[:, :, :, DQ : DQ + 1], 1.0)

        wts = {"Wq": WqT, "Wk": WkT, "Wv": WvT, "Wo": WoT}

        # ---- Stage A/B: transpose weights and x into SBUF (PE transpose) ----
        with (
            tc.tile_pool(name="ld", bufs=3) as ld,
            tc.tile_pool(name="tp", bufs=8, space="PSUM") as tp,
        ):
            for name, wt in wts.items():
                for jt in range(4):
                    wblk = ld.tile([128, D], F32, tag="blk")
                    nc.sync.dma_start(out=wblk, in_=ws[name][jt * 128 : (jt + 1) * 128, :])
                    for dc in range(4):
                        pt = tp.tile([128, 128], F32, tag="tr")
                        nc.tensor.transpose(pt, wblk[:, dc * 128 : (dc + 1) * 128], identity)
                        nc.vector.tensor_copy(out=wt[:, dc, jt * 128 : (jt + 1) * 128], in_=pt)
            for tt in range(T // 128):
                xblk = ld.tile([128, D], F32, tag="blk")
                nc.sync.dma_start(out=xblk, in_=xkv[tt * 128 : (tt + 1) * 128, :])
                for dc in range(4):
                    pt = tp.tile([128, 128], F32, tag="tr")
                    nc.tensor.transpose(pt, xblk[:, dc * 128 : (dc + 1) * 128], identity)
                    nc.vector.tensor_copy(out=XT[:, dc, tt * 128 : (tt + 1) * 128], in_=pt)
            for tt in range(TQ // 128):
                xblk = ld.tile([128, D], F32, tag="blk")
                nc.sync.dma_start(out=xblk, in_=xq[tt * 128 : (tt + 1) * 128, :])
                for dc in range(4):
                    pt = tp.tile([128, 128], F32, tag="tr")
                    nc.tensor.transpose(pt, xblk[:, dc * 128 : (dc + 1) * 128], identity)
                    nc.vector.tensor_copy(out=XQT[:, dc, tt * 128 : (tt + 1) * 128], in_=pt)

        # ---- Stage C: projections ----
        with tc.tile_pool(name="pp", bufs=6, space="PSUM") as pp:
            # K.T[j, t] and Q.T[j, t]: lhsT = W.T[d, j-tile], rhs = X.T[d, t-chunk]
            for jt in range(4):
                for ch in range(T // 512):
                    kp = pp.tile([128, 512], F32, tag="proj")
                    for dc in range(4):
                        nc.tensor.matmul(
                            kp,
                            lhsT=WkT[:, dc, jt * 128 : (jt + 1) * 128],
                            rhs=XT[:, dc, ch * 512 : (ch + 1) * 512],
                            start=(dc == 0),
                            stop=(dc == 3),
                        )
                    nc.vector.tensor_scalar_add(
                        out=KT[:, jt, ch * 512 : (ch + 1) * 512],
                        in0=kp,
                        scalar1=bk_s[:, jt : jt + 1],
                    )
                for ch in range(TQ // 512):
                    qp = pp.tile([128, 512], F32, tag="proj")
                    for dc in range(4):
                        nc.tensor.matmul(
                            qp,
                            lhsT=WqT[:, dc, jt * 128 : (jt + 1) * 128],
                            rhs=XQT[:, dc, ch * 512 : (ch + 1) * 512],
                            start=(dc == 0),
                            stop=(dc == 3),
                        )
                    nc.vector.tensor_scalar_add(
                        out=QT[:, jt, ch * 512 : (ch + 1) * 512],
                        in0=qp,
                        scalar1=bq_s[:, jt : jt + 1],
                    )
            # V[t, j]: lhsT = X.T[d, t-tile], rhs = Wv.T[d, :]
            for tt in range(T // 128):
                vp = pp.tile([128, 512], F32, tag="proj")
                for dc in range(4):
                    nc.tensor.matmul(
                        vp,
                        lhsT=XT[:, dc, tt * 128 : (tt + 1) * 128],
                        rhs=WvT[:, dc, :],
                        start=(dc == 0),
                        stop=(dc == 3),
                    )
                nc.vector.tensor_add(
                    out=VH[:, tt, :, 0:DQ],
                    in0=vp.rearrange("p (h v) -> p h v", h=H),
                    in1=bv_b.rearrange("p (h v) -> p h v", h=H),
                )

        # ---- Stage D: attention ----
        ngroups = (T // 128 + G - 1) // G
        with (
            tc.tile_pool(name="sp", bufs=3, space="PSUM") as sp,
            tc.tile_pool(name="op", bufs=2, space="PSUM") as op,
            tc.tile_pool(name="se", bufs=4) as se,
            tc.tile_pool(name="epi", bufs=4) as epi,
        ):
            for qc in range(TQ // 512):
                qsl = slice(qc * 512, (qc + 1) * 512)
                for hp in range(NHP):
                    attO = [
                        op.tile([DQ + 1, 512], F32, tag="attO", name=f"attO{p}")
                        for p in range(2)
                    ]
                    tkt = 0
                    for g in range(ngroups):
                        gsz = min(G, T // 128 - tkt)
                        sc = [
                            sp.tile([128, G, 512], F32, tag="sc", name=f"sc{p}")
                            for p in range(2)
                        ]
                        # interleave even/odd head score matmuls: adjacent
                        # K=64 matmuls on disjoint PE row-groups overlap.
                        # Chain them with no-sync dep edges so the Tile
                        # scheduler preserves the alternation.
                        for i in range(gsz):
                            tksl = slice((tkt + i) * 128, (tkt + i + 1) * 128)
                            mm_pair = []
                            for par in range(2):
                                psl = slice(par * 64, (par + 1) * 64)
                                mm = nc.tensor.matmul(
                                    sc[par][:, i, :],
                                    lhsT=KT[psl, hp, tksl],
                                    rhs=QT[psl, hp, qsl],
                                    start=True,
                                    stop=True,
                                )
                                mm_pair.append(mm)
                            tile.add_dep_helper(
                                mm_pair[1].ins, mm_pair[0].ins, False, "pair order"
                            )
                        for par in range(2):
                            h = hp * 2 + par
                            ex = se.tile([128, G, 512], BF16, tag="ex")
                            nc.scalar.activation(
                                out=ex[:, :gsz, :],
                                in_=sc[par][:, :gsz, :],
                                func=mybir.ActivationFunctionType.Exp,
                                scale=float(DQ) ** -0.5,
                            )
                            for i in range(gsz):
                                nc.tensor.matmul(
                                    attO[par],
                                    lhsT=VH[:, tkt + i, h, :],
                                    rhs=ex[:, i, :],
                                    start=(tkt + i == 0),
                                    stop=(tkt + i == T // 128 - 1),
                                    skip_group_check=True,
                                )
                        tkt += gsz
                    for par in range(2):
                        rden = epi.tile([1, 512], F32, tag="rden")
                        nc.vector.reciprocal_approx_fast(
                            out=rden, in_=attO[par][DQ : DQ + 1, :]
                        )
                        bc = epi.tile([64, 512], F32, tag="bc")
                        nc.gpsimd.partition_broadcast(out_ap=bc, in_ap=rden)
                        nc.vector.tensor_mul(
                            out=AOT[par * 64 : (par + 1) * 64, hp, qsl],
                            in0=attO[par][0:DQ, :],
                            in1=bc,
                        )

        # ---- Stage E: output projection ----
        with (
            tc.tile_pool(name="yp", bufs=4, space="PSUM") as yp,
            tc.tile_pool(name="yo", bufs=4) as yo,
        ):
            for tt in range(TQ // 128):
                ypt = yp.tile([128, 512], F32, tag="y")
                for hp in range(NHP):
                    nc.tensor.matmul(
                        ypt,
                        lhsT=AOT[:, hp, tt * 128 : (tt + 1) * 128],
                        rhs=WoT[:, hp, :],
                        start=(hp == 0),
                        stop=(hp == NHP - 1),
                    )
                ys = yo.tile([128, 512], F32, tag="ys")
                nc.vector.tensor_add(out=ys, in0=ypt, in1=bo_b)
                nc.sync.dma_start(out=y[tt * 128 : (tt + 1) * 128, :], in_=ys)


_CACHED = {}


def _get_program():
    if "nc" not in _CACHED:
        _CACHED["nc"] = _build_program()
    return _CACHED["nc"]


def kernel(x, Wq, bq, Wk, bk, Wv, bv, Wo, bo, _trace=False):
    x = np.ascontiguousarray(np.asarray(x, dtype=np.float32))
    weights = {
        "Wq": np.ascontiguousarray(np.asarray(Wq, dtype=np.float32)),
        "Wk": np.ascontiguousarray(np.asarray(Wk, dtype=np.float32)),
        "Wv": np.ascontiguousarray(np.asarray(Wv, dtype=np.float32)),
        "Wo": np.ascontiguousarray(np.asarray(Wo, dtype=np.float32)),
        "bq": np.ascontiguousarray(np.asarray(bq, dtype=np.float32)),
        "bk": np.ascontiguousarray(np.asarray(bk, dtype=np.float32)),
        "bv": np.ascontiguousarray(np.asarray(bv, dtype=np.float32)),
        "bo": np.ascontiguousarray(np.asarray(bo, dtype=np.float32)),
    }

    nc = _get_program()
    in_maps = []
    for c in range(NCORES):
        b = c // 4
        q0 = (c % 4) * TQ
        in_maps.append(
            {"xkv": x[b], "xq": x[b, q0 : q0 + TQ], **weights}
        )

    res = run_bass_kernel_spmd(
        nc, in_maps, core_ids=list(range(NCORES)), trace=_trace
    )

    out = np.empty((B, T, D), dtype=np.float32)
    for c in range(NCORES):
        b = c // 4
        q0 = (c % 4) * TQ
        out[b, q0 : q0 + TQ] = res.results[c]["y"]
    if _trace:
        return out, res
    return out
